# revision 7
# baseline (speedup 1.0000x reference)
"""Bass/Trainium2 kernel for batched GNN message passing:
    out[b, d, n] = sum_m adj[b, n, m] * x[b, d, m]
B=2, D=3072, N=8192, fp32 in/out.

Sharding: 8 cores, core c -> (b = c//4, n-quarter = c%4). Each core computes
C[3072, 2048] = X[b] @ A[b, quarter, :].T with contraction m = 8192.

Strategy (bf16, zero on-chip transposes, zero DRAM partials):
- Host prepacks both operands transposed + tiled so every DMA is contiguous
  and every matmul operand is already in [contraction-on-partitions] layout.
  bf16 rounding gives rel err ~2e-3 vs the 2e-2 gate (measured on the real
  seeded inputs).
- Per core: 4 n-slabs of 512 cols. Per slab, the full-contraction adj panel
  [128k x 64mc x 512n] (64 KiB/partition) is SBUF-resident (double-buffered
  across slabs -> no PE stall at slab swap). For each of 24 d-blocks, one
  PSUM bank accumulates out[128d, 512n] over all 64 mc chunks in a dense
  back-to-back matmul stream (LDWEIGHTS hides in the PE reorder window, HAM
  stays warm). X d-block strips re-stream per slab (4x50MB, hidden under
  compute).
- Evict: PSUM -> VectorE copy -> SBUF -> DMA out. Panel DMAs issue from the
  ScalarE HWDGE queue so they prefetch ahead of the x/out Sync-queue traffic.
"""

import sys
from contextlib import ExitStack

import numpy as np

sys.path.insert(0, "/opt/trn_rl_repo")

B = 2
D = 3072
N = 8192
NCORES = 8
NSPLIT = 4  # n-quarters per batch sample
NC = N // NSPLIT  # 2048 columns of out per core

P = 128
NDB = D // P  # 24 d-blocks
NMC = N // P  # 64 contraction chunks
NSLAB = 4  # n-slabs per core
NW = NC // NSLAB  # 512 cols per slab


def build_program():
    """Build the per-core Bass program. Returns compiled nc."""
    import concourse.mybir as mybir
    import concourse.tile as tile
    from concourse import bacc

    f32 = mybir.dt.float32
    bf16 = mybir.dt.bfloat16

    nc = bacc.Bacc(None, target_bir_lowering=False, debug=False)

    # xh[db*128 + k, mc*128 + i] = x[b][db*128 + i, mc*128 + k]  (bf16)
    xh = nc.dram_tensor("xh", [D, NMC * P], bf16, kind="ExternalInput")
    # ah[ns*128 + k, mc*512 + j] = adj[b][q*2048 + ns*512 + j, mc*128 + k]
    ah = nc.dram_tensor("ah", [NSLAB * P, NMC * NW], bf16, kind="ExternalInput")
    out_ext = nc.dram_tensor("out", [D, NC], f32, kind="ExternalOutput")

    with tile.TileContext(nc) as tc, ExitStack() as ctx:
        panel_pool = ctx.enter_context(tc.tile_pool(name="panel", bufs=2))
        x_pool = ctx.enter_context(tc.tile_pool(name="xp", bufs=3))
        out_pool = ctx.enter_context(tc.tile_pool(name="outp", bufs=4))
        acc_psum = ctx.enter_context(tc.tile_pool(name="accp", bufs=4, space="PSUM"))

        # DMAs are split into pieces: Tile tracks sub-range deps, so matmuls
        # start as soon as the first chunk lands instead of stalling ~30us on
        # the full panel transfer. Tensor-engine instructions execute in
        # program order, so emission order is the PE schedule.

        def load_x(db, piece_mcs):
            """piece_mcs: list of mc-chunk counts per DMA piece (sums to NMC)."""
            xs = x_pool.tile([P, NMC * P], bf16, tag="xs")
            lo = 0
            for n in piece_mcs:
                nc.sync.dma_start(
                    out=xs[:, lo * P : (lo + n) * P],
                    in_=xh[db * P : (db + 1) * P, lo * P : (lo + n) * P],
                )
                lo += n
            return xs

        def mm_group(acc, xs, panel, mcs):
            for mc in mcs:
                nc.tensor.matmul(
                    acc[:],
                    xs[:, mc * P : (mc + 1) * P],
                    panel[:, mc * NW : (mc + 1) * NW],
                    start=(mc == 0),
                    stop=(mc == NMC - 1),
                )

        def evict(acc, db, ns):
            osb = out_pool.tile([P, NW], f32, tag="osb")
            nc.vector.tensor_copy(out=osb[:], in_=acc[:])
            nc.sync.dma_start(
                out=out_ext[db * P : (db + 1) * P, ns * NW : (ns + 1) * NW],
                in_=osb[:],
            )

        for ns in range(NSLAB):
            panel = panel_pool.tile([P, NMC * NW], bf16, tag="panel")
            # ScalarE HWDGE queue: prefetches ahead of the sync-queue traffic.
            # Graduated piece sizes (in mc chunks) for slab 0 so the first
            # matmuls start within a couple of us.
            piece_mcs = [1, 1, 2, 4, 8, 8, 8, 8, 8, 8, 8] if ns == 0 else [8] * 8
            lo = 0
            for n in piece_mcs:
                nc.scalar.dma_start(
                    out=panel[:, lo * NW : (lo + n) * NW],
                    in_=ah[ns * P : (ns + 1) * P, lo * NW : (lo + n) * NW],
                )
                lo += n

            if ns == 0:
                # Startup: panel-0 streams in at HBM pace (~25us), slower than
                # one d-block's matmuls (13.7us). Interleave the first three
                # d-blocks piece-by-piece so the PE stays busy throughout.
                NI = 3
                xss = [load_x(db, [2, 6, 24, 32]) for db in range(NI)]
                accs = [
                    acc_psum.tile([P, NW], f32, tag="acc", name=f"acc{i}")
                    for i in range(NI)
                ]
                lo = 0
                for n in piece_mcs:
                    for db in range(NI):
                        mm_group(accs[db], xss[db], panel, range(lo, lo + n))
                    lo += n
                for db in range(NI):
                    evict(accs[db], db, ns)
                rest = range(NI, NDB)
            else:
                rest = range(NDB)

            for db in rest:
                xs = load_x(db, [32, 32])
                acc = acc_psum.tile([P, NW], f32, tag="acc")
                mm_group(acc, xs, panel, range(NMC))
                evict(acc, db, ns)

    nc.compile()
    return nc


_NC_CACHE = {}


def _get_program():
    if "nc" not in _NC_CACHE:
        _NC_CACHE["nc"] = build_program()
    return _NC_CACHE["nc"]


def prepare_in_maps(x: np.ndarray, adj: np.ndarray) -> list:
    """Host-side prepack: transpose + tile + bf16-cast both operands."""
    import ml_dtypes

    bf16 = ml_dtypes.bfloat16

    xh_by_b = []
    for b in range(B):
        # [D, M] -> XT [M, D] bf16 -> [mc, k, db, i] -> [db, k, mc, i]
        xt = x[b].T.astype(bf16)  # [8192, 3072] contiguous copy
        xh = (
            xt.reshape(NMC, P, NDB, P)
            .transpose(2, 1, 0, 3)
            .reshape(D, NMC * P)
        )
        xh_by_b.append(np.ascontiguousarray(xh))

    in_maps = []
    for c in range(NCORES):
        b, q = divmod(c, NSPLIT)
        a = adj[b, q * NC : (q + 1) * NC, :].astype(bf16)  # [2048, 8192]
        # [ns, j, mc, k] -> [ns, k, mc, j]
        ah = (
            a.reshape(NSLAB, NW, NMC, P)
            .transpose(0, 3, 2, 1)
            .reshape(NSLAB * P, NMC * NW)
        )
        in_maps.append({"xh": xh_by_b[b], "ah": np.ascontiguousarray(ah)})
    return in_maps


def kernel(x: np.ndarray, adj: np.ndarray) -> np.ndarray:
    """Full inputs in, full output out. x [B,D,N] f32, adj [B,N,N] f32."""
    from concourse.bass_utils import run_bass_kernel_spmd

    assert x.shape == (B, D, N) and adj.shape == (B, N, N)
    nc = _get_program()
    in_maps = prepare_in_maps(np.asarray(x), np.asarray(adj))

    res = run_bass_kernel_spmd(nc, in_maps, core_ids=list(range(NCORES)))
    out = np.empty((B, D, N), dtype=np.float32)
    for c in range(NCORES):
        b, q = divmod(c, NSPLIT)
        out[b, :, q * NC : (q + 1) * NC] = res.results[c]["out"]
    return out


# revision 10
# speedup vs baseline: 1.0087x; 1.0087x over previous
"""Bass/Trainium2 kernel for batched GNN message passing:
    out[b, d, n] = sum_m adj[b, n, m] * x[b, d, m]
B=2, D=3072, N=8192, fp32 in/out.

Sharding: 8 cores, core c -> (b = c//4, n-quarter = c%4). Each core computes
C[3072, 2048] = X[b] @ A[b, quarter, :].T with contraction m = 8192.

Strategy (bf16, zero on-chip transposes, zero DRAM partials):
- Host prepacks both operands transposed + tiled so every DMA is contiguous
  and every matmul operand is already in [contraction-on-partitions] layout.
  bf16 rounding gives rel err ~2e-3 vs the 2e-2 gate (measured on the real
  seeded inputs).
- Per core: 4 n-slabs of 512 cols. Per slab, the full-contraction adj panel
  [128k x 64mc x 512n] (64 KiB/partition) is SBUF-resident (double-buffered
  across slabs -> no PE stall at slab swap). For each of 24 d-blocks, one
  PSUM bank accumulates out[128d, 512n] over all 64 mc chunks in a dense
  back-to-back matmul stream (LDWEIGHTS hides in the PE reorder window, HAM
  stays warm). X d-block strips re-stream per slab (4x50MB, hidden under
  compute).
- Evict: PSUM -> VectorE copy -> SBUF -> DMA out. Panel DMAs issue from the
  ScalarE HWDGE queue so they prefetch ahead of the x/out Sync-queue traffic.
"""

import sys
from contextlib import ExitStack

import numpy as np

sys.path.insert(0, "/opt/trn_rl_repo")

B = 2
D = 3072
N = 8192
NCORES = 8
NSPLIT = 4  # n-quarters per batch sample
NC = N // NSPLIT  # 2048 columns of out per core

P = 128
NDB = D // P  # 24 d-blocks
NMC = N // P  # 64 contraction chunks
NSLAB = 4  # n-slabs per core
NW = NC // NSLAB  # 512 cols per slab


def build_program():
    """Build the per-core Bass program. Returns compiled nc."""
    import concourse.mybir as mybir
    import concourse.tile as tile
    from concourse import bacc

    f32 = mybir.dt.float32
    bf16 = mybir.dt.bfloat16

    nc = bacc.Bacc(None, target_bir_lowering=False, debug=False)

    # xh[db*128 + k, mc*128 + i] = x[b][db*128 + i, mc*128 + k]  (bf16)
    xh = nc.dram_tensor("xh", [D, NMC * P], bf16, kind="ExternalInput")
    # ah[ns*128 + k, mc*512 + j] = adj[b][q*2048 + ns*512 + j, mc*128 + k]
    ah = nc.dram_tensor("ah", [NSLAB * P, NMC * NW], bf16, kind="ExternalInput")
    out_ext = nc.dram_tensor("out", [D, NC], f32, kind="ExternalOutput")

    with tile.TileContext(nc) as tc, ExitStack() as ctx:
        panel_pool = ctx.enter_context(tc.tile_pool(name="panel", bufs=2))
        x_pool = ctx.enter_context(tc.tile_pool(name="xp", bufs=4))
        out_pool = ctx.enter_context(tc.tile_pool(name="outp", bufs=4))
        acc_psum = ctx.enter_context(tc.tile_pool(name="accp", bufs=4, space="PSUM"))

        # DMAs are split into pieces: Tile tracks sub-range deps, so matmuls
        # start as soon as the first chunk lands instead of stalling ~30us on
        # the full panel transfer. Tensor-engine instructions execute in
        # program order, so emission order is the PE schedule.

        def load_x(db, piece_mcs):
            """piece_mcs: list of mc-chunk counts per DMA piece (sums to NMC)."""
            xs = x_pool.tile([P, NMC * P], bf16, tag="xs")
            lo = 0
            for n in piece_mcs:
                nc.sync.dma_start(
                    out=xs[:, lo * P : (lo + n) * P],
                    in_=xh[db * P : (db + 1) * P, lo * P : (lo + n) * P],
                )
                lo += n
            return xs

        def mm_group(acc, xs, panel, mcs):
            for mc in mcs:
                nc.tensor.matmul(
                    acc[:],
                    xs[:, mc * P : (mc + 1) * P],
                    panel[:, mc * NW : (mc + 1) * NW],
                    start=(mc == 0),
                    stop=(mc == NMC - 1),
                )

        def evict(acc, db, ns):
            osb = out_pool.tile([P, NW], f32, tag="osb")
            nc.vector.tensor_copy(out=osb[:], in_=acc[:])
            nc.sync.dma_start(
                out=out_ext[db * P : (db + 1) * P, ns * NW : (ns + 1) * NW],
                in_=osb[:],
            )

        for ns in range(NSLAB):
            panel = panel_pool.tile([P, NMC * NW], bf16, tag="panel")
            # ScalarE HWDGE queue: prefetches ahead of the sync-queue traffic.
            # Graduated piece sizes (in mc chunks) for slab 0 so the first
            # matmuls start within a couple of us.
            piece_mcs = [1, 1, 2, 4, 8, 8, 8, 8, 8, 8, 8] if ns == 0 else [8] * 8
            lo = 0
            for n in piece_mcs:
                nc.scalar.dma_start(
                    out=panel[:, lo * NW : (lo + n) * NW],
                    in_=ah[ns * P : (ns + 1) * P, lo * NW : (lo + n) * NW],
                )
                lo += n

            if ns == 0:
                # Startup: panel-0 streams in at HBM pace (~25us), slower than
                # one d-block's matmuls (13.7us). Interleave the first three
                # d-blocks piece-by-piece so the PE stays busy throughout.
                # x pieces are emitted round-robin across the three tiles so
                # every tile's first chunk lands before any tile's bulk.
                NI = 3
                xss = [
                    x_pool.tile([P, NMC * P], bf16, tag="xs", name=f"xsi{i}")
                    for i in range(NI)
                ]
                x_piece_mcs = [2, 6, 8, 16, 32]
                lo = 0
                for n in x_piece_mcs:
                    for db in range(NI):
                        nc.sync.dma_start(
                            out=xss[db][:, lo * P : (lo + n) * P],
                            in_=xh[db * P : (db + 1) * P, lo * P : (lo + n) * P],
                        )
                    lo += n
                accs = [
                    acc_psum.tile([P, NW], f32, tag="acc", name=f"acc{i}")
                    for i in range(NI)
                ]
                lo = 0
                for n in piece_mcs:
                    for db in range(NI):
                        mm_group(accs[db], xss[db], panel, range(lo, lo + n))
                    lo += n
                for db in range(NI):
                    evict(accs[db], db, ns)
                rest = range(NI, NDB)
            else:
                rest = range(NDB)

            for db in rest:
                xs = load_x(db, [32, 32])
                acc = acc_psum.tile([P, NW], f32, tag="acc")
                mm_group(acc, xs, panel, range(NMC))
                evict(acc, db, ns)

    nc.compile()
    return nc


_NC_CACHE = {}


def _get_program():
    if "nc" not in _NC_CACHE:
        _NC_CACHE["nc"] = build_program()
    return _NC_CACHE["nc"]


def prepare_in_maps(x: np.ndarray, adj: np.ndarray) -> list:
    """Host-side prepack: transpose + tile + bf16-cast both operands."""
    import ml_dtypes

    bf16 = ml_dtypes.bfloat16

    xh_by_b = []
    for b in range(B):
        # [D, M] -> XT [M, D] bf16 -> [mc, k, db, i] -> [db, k, mc, i]
        xt = x[b].T.astype(bf16)  # [8192, 3072] contiguous copy
        xh = (
            xt.reshape(NMC, P, NDB, P)
            .transpose(2, 1, 0, 3)
            .reshape(D, NMC * P)
        )
        xh_by_b.append(np.ascontiguousarray(xh))

    in_maps = []
    for c in range(NCORES):
        b, q = divmod(c, NSPLIT)
        a = adj[b, q * NC : (q + 1) * NC, :].astype(bf16)  # [2048, 8192]
        # [ns, j, mc, k] -> [ns, k, mc, j]
        ah = (
            a.reshape(NSLAB, NW, NMC, P)
            .transpose(0, 3, 2, 1)
            .reshape(NSLAB * P, NMC * NW)
        )
        in_maps.append({"xh": xh_by_b[b], "ah": np.ascontiguousarray(ah)})
    return in_maps


def kernel(x: np.ndarray, adj: np.ndarray) -> np.ndarray:
    """Full inputs in, full output out. x [B,D,N] f32, adj [B,N,N] f32."""
    from concourse.bass_utils import run_bass_kernel_spmd

    assert x.shape == (B, D, N) and adj.shape == (B, N, N)
    nc = _get_program()
    in_maps = prepare_in_maps(np.asarray(x), np.asarray(adj))

    res = run_bass_kernel_spmd(nc, in_maps, core_ids=list(range(NCORES)))
    out = np.empty((B, D, N), dtype=np.float32)
    for c in range(NCORES):
        b, q = divmod(c, NSPLIT)
        out[b, :, q * NC : (q + 1) * NC] = res.results[c]["out"]
    return out
